# revision 5
# baseline (speedup 1.0000x reference)
"""Trainium2 Bass kernel for nn_Attention_36601711297049.

Self-attention (4 heads, dim_head 32) over N=4096 tokens, batch 2:
  qkv = w_qkv @ x ; sim = scale * q^T k ; attn = softmax(sim) ;
  out = attn @ v ; y = w_out @ out + b_out

Sharding: 8 cores = 2 batches x 4 query-chunks (1024 queries each).
Each core computes k, v for the full batch plus q for its own chunk, runs
flash-style attention in S^T layout ([keys, queries], so the AV contraction
needs no transposes), and applies the output projection locally. No
collectives. Softmax skips max-subtraction (logits are ~N(0,1), safely
inside fp32/exp range).

Per core engine budget: ScalarE exp (the only exp engine) dominates at
~129us; PE ~60us of bf16 matmuls (4-way row-packed S^T, col-packed AV and
denominator passes) hides underneath.
"""
import sys

for p in ("/opt/trn_rl_repo",):
    if p not in sys.path:
        sys.path.insert(0, p)

import numpy as np
from contextlib import ExitStack

import concourse.bass as bass
from concourse import bacc
import concourse.tile as tile
from concourse import mybir
from concourse.bass_utils import run_bass_kernel_spmd

F32 = mybir.dt.float32
BF16 = mybir.dt.bfloat16
AF = mybir.ActivationFunctionType

HEADS = 4
DH = 32
C = 256          # channels
N = 4096         # h*w tokens per batch
QC = 1024        # queries per core
NQ = QC // 512   # 512-query chunks per core
NK = N // 128    # 128-key tiles
SCALE = float(DH) ** -0.5


def build_nc():
    nc = bacc.Bacc("TRN2", target_bir_lowering=False)
    x = nc.dram_tensor("x", [C, N], F32, kind="ExternalInput")
    xq = nc.dram_tensor("xq", [C, QC], F32, kind="ExternalInput")
    wq = nc.dram_tensor("wq", [C, 128], F32, kind="ExternalInput")   # w_q^T, rows (h,d)
    wk = nc.dram_tensor("wk", [C, 128], F32, kind="ExternalInput")
    wv = nc.dram_tensor("wv", [C, 128], F32, kind="ExternalInput")
    wo = nc.dram_tensor("wo", [128, C], F32, kind="ExternalInput")   # w_out^T
    bo = nc.dram_tensor("bo", [C], F32, kind="ExternalInput")
    out = nc.dram_tensor("out", [C, QC], F32, kind="ExternalOutput")

    with tile.TileContext(nc) as tc, ExitStack() as ctx:
        big = ctx.enter_context(tc.tile_pool(name="big", bufs=1))
        small = ctx.enter_context(tc.tile_pool(name="small", bufs=2))
        ptp = ctx.enter_context(tc.tile_pool(name="ptp", bufs=3))
        stp = ctx.enter_context(tc.tile_pool(name="stp", bufs=3, space="PSUM"))
        avp = ctx.enter_context(tc.tile_pool(name="avp", bufs=1, space="PSUM"))
        dyp = ctx.enter_context(tc.tile_pool(name="dyp", bufs=1, space="PSUM"))

        # ---- constants / weights ----
        wq_bf = big.tile([128, 2, 128], BF16, tag="wq_bf")
        wk_bf = big.tile([128, 2, 128], BF16, tag="wk_bf")
        wv_bf = big.tile([128, 2, 128], BF16, tag="wv_bf")
        for (dram, sbuf) in ((wq, wq_bf), (wk, wk_bf), (wv, wv_bf)):
            st = big.tile([128, 2, 128], F32, tag="w_stage", name=f"st_{sbuf.name}",
                          bufs=3)
            nc.sync.dma_start(st[:], dram.rearrange("(cc p) o -> p cc o", p=128))
            nc.vector.tensor_copy(sbuf[:], st[:])
        wo_f = big.tile([128, 256], F32, tag="wo_f")
        wo_bf = big.tile([128, 256], BF16, tag="wo_bf")
        nc.sync.dma_start(wo_f[:], wo[:])
        nc.vector.tensor_copy(wo_bf[:], wo_f[:])
        bias_sb = big.tile([128, 2], F32, tag="bias_sb")
        nc.sync.dma_start(bias_sb[:], bo.rearrange("(oc p) -> p oc", p=128))
        ones_bf = big.tile([128, DH], BF16, tag="ones_bf")
        nc.vector.memset(ones_bf[:], 1.0)

        # warm the exp table set early (one tiny ACT forces the table load)
        dummy = small.tile([1, 8], F32, tag="dummy")
        nc.vector.memset(dummy[:], 0.0)
        nc.scalar.activation(dummy[:], dummy[:], AF.Exp)

        # ---- load x / xq, convert to bf16 ----
        # DMAs spread across engine queues; n-major piece order so the k GEMM
        # (which needs both c-chunks of a given n-range) can start early.
        x_f = big.tile([128, 2, N], F32, tag="x_f")
        x_bf = big.tile([128, 2, N], BF16, tag="x_bf")
        xq_f = big.tile([128, 2, QC], F32, tag="xq_f")
        xq_bf = big.tile([128, 2, QC], BF16, tag="xq_bf")
        dma_engines = (nc.sync, nc.gpsimd, nc.scalar)
        for cc in range(2):
            dma_engines[cc % 3].dma_start(xq_f[:, cc, :], xq[128 * cc:128 * (cc + 1), :])
            nc.vector.tensor_copy(xq_bf[:, cc, :], xq_f[:, cc, :])
        di = 0
        for piece in range(4):
            sl = slice(1024 * piece, 1024 * (piece + 1))
            for cc in range(2):
                dma_engines[di % 3].dma_start(
                    x_f[:, cc, sl], x[128 * cc:128 * (cc + 1), sl])
                di += 1
                nc.vector.tensor_copy(x_bf[:, cc, sl], x_f[:, cc, sl])

        # ---- q = wq^T x_q : [128 (h,d), QC] bf16 ----
        q_bf = big.tile([128, QC], BF16, tag="q_bf")
        for nch in range(NQ):
            ps = stp.tile([128, 1024], F32, tag="st", name=f"q_ps{nch}")
            for cc in range(2):
                nc.tensor.matmul(ps[:, :512], wq_bf[:, cc, :],
                                 xq_bf[:, cc, 512 * nch:512 * (nch + 1)],
                                 start=(cc == 0), stop=(cc == 1),
                                 skip_group_check=True)
            nc.vector.tensor_copy(q_bf[:, 512 * nch:512 * (nch + 1)], ps[:, :512])

        # ---- k = wk^T x and vT = x^T wv, emitted piecewise and interleaved
        # into the first query-chunk's attention loop so the exp stream
        # starts as soon as the first x piece lands ----
        k_bf = big.tile([128, N], BF16, tag="k_bf")
        vT_bf = big.tile([128, N], BF16, tag="vT_bf")

        def emit_k_gemm(nch):
            ps = stp.tile([128, 1024], F32, tag="st", name=f"k_ps{nch}")
            for cc in range(2):
                nc.tensor.matmul(ps[:, :512], wk_bf[:, cc, :],
                                 x_bf[:, cc, 512 * nch:512 * (nch + 1)],
                                 start=(cc == 0), stop=(cc == 1),
                                 skip_group_check=True)
            nc.vector.tensor_copy(k_bf[:, 512 * nch:512 * (nch + 1)], ps[:, :512])

        def emit_vT_gemm(kt):
            ps = stp.tile([128, 1024], F32, tag="st", name=f"v_ps{kt}")
            for cc in range(2):
                nc.tensor.matmul(ps[:, :128], x_bf[:, cc, 128 * kt:128 * (kt + 1)],
                                 wv_bf[:, cc, :],
                                 start=(cc == 0), stop=(cc == 1),
                                 skip_group_check=True)
            nc.vector.tensor_copy(vT_bf[:, 128 * kt:128 * (kt + 1)], ps[:, :128])

        # ---- attention main loop ----
        for qc in range(NQ):
            qsl = slice(512 * qc, 512 * (qc + 1))
            av = avp.tile([128, 512], F32, tag="acc", name=f"av{qc}")
            den = dyp.tile([128, 512], F32, tag="dy", name=f"den{qc}")
            for kt in range(NK):
                if qc == 0:
                    if kt % 4 == 0:
                        emit_k_gemm(kt // 4)
                    emit_vT_gemm(kt)
                # S^T: 4 heads row-packed, 2 heads per psum tile
                st0 = stp.tile([128, 1024], F32, tag="st", name=f"st0_{qc}_{kt}")
                st1 = stp.tile([128, 1024], F32, tag="st", name=f"st1_{qc}_{kt}")
                sts = (st0, st0, st1, st1)
                for h in range(HEADS):
                    nc.tensor.matmul(
                        sts[h][:, 512 * (h % 2):512 * (h % 2 + 1)],
                        k_bf[32 * h:32 * (h + 1), 128 * kt:128 * (kt + 1)],
                        q_bf[32 * h:32 * (h + 1), qsl],
                        start=True, stop=True, skip_group_check=True,
                        tile_position=(32 * h, 0))
                # exp (scale folded in), psum -> sbuf bf16
                pt0 = ptp.tile([128, 1024], BF16, tag="pt", name=f"pt0_{qc}_{kt}")
                pt1 = ptp.tile([128, 1024], BF16, tag="pt", name=f"pt1_{qc}_{kt}")
                nc.scalar.activation(pt0[:], st0[:], AF.Exp, scale=SCALE)
                nc.scalar.activation(pt1[:], st1[:], AF.Exp, scale=SCALE)
                pts = (pt0, pt0, pt1, pt1)
                # AV + denominator, col-packed by head
                for h in range(HEADS):
                    psl = slice(512 * (h % 2), 512 * (h % 2 + 1))
                    nc.tensor.matmul(
                        av[32 * h:32 * (h + 1), :],
                        vT_bf[:, 128 * kt + 32 * h:128 * kt + 32 * (h + 1)],
                        pts[h][:, psl],
                        start=(kt == 0), stop=(kt == NK - 1),
                        skip_group_check=True, tile_position=(0, 32 * h))
                for h in range(HEADS):
                    psl = slice(512 * (h % 2), 512 * (h % 2 + 1))
                    nc.tensor.matmul(
                        den[32 * h:32 * (h + 1), :],
                        ones_bf[:],
                        pts[h][:, psl],
                        start=(kt == 0), stop=(kt == NK - 1),
                        skip_group_check=True, tile_position=(0, 32 * h))

            # normalize: hidden = av / den  (den rows replicated per head)
            rec = small.tile([128, 512], F32, tag="rec", name=f"rec{qc}")
            nc.vector.reciprocal(rec[:], den[:])
            hid = small.tile([128, 512], BF16, tag="hid", name=f"hid{qc}")
            nc.vector.tensor_mul(hid[:], av[:], rec[:])

            # output projection + bias
            for oc in range(2):
                yps = dyp.tile([128, 512], F32, tag="dy", name=f"y{qc}_{oc}")
                nc.tensor.matmul(yps[:], wo_bf[:, 128 * oc:128 * (oc + 1)], hid[:],
                                 start=True, stop=True, skip_group_check=True)
                ysb = small.tile([128, 512], F32, tag="ysb", name=f"ysb{qc}_{oc}")
                nc.vector.tensor_add(ysb[:], yps[:],
                                     bias_sb[:, oc:oc + 1].broadcast_to([128, 512]))
                nc.sync.dma_start(out[128 * oc:128 * (oc + 1), qsl], ysb[:])
    return nc


_NC_CACHE = None


def _get_nc():
    global _NC_CACHE
    if _NC_CACHE is None:
        nc = build_nc()
        nc.compile()
        _NC_CACHE = nc
    return _NC_CACHE


def _prep_weights(w_qkv, w_out, b_out):
    # w_qkv rows are interleaved: row (h*32+d)*3 + {0:q, 1:k, 2:v}
    w = np.asarray(w_qkv, np.float32).reshape(HEADS, DH, 3, C)
    wq = np.ascontiguousarray(w[:, :, 0, :].reshape(128, C).T)   # [C, 128]
    wk = np.ascontiguousarray(w[:, :, 1, :].reshape(128, C).T)
    wv = np.ascontiguousarray(w[:, :, 2, :].reshape(128, C).T)
    wo = np.ascontiguousarray(np.asarray(w_out, np.float32).T)   # [128, C]
    bo = np.asarray(b_out, np.float32)
    return wq, wk, wv, wo, bo


def kernel(x, w_qkv, w_out, b_out):
    x = np.asarray(x, np.float32)
    b, c, h, w = x.shape
    hw = h * w
    xf = np.ascontiguousarray(x.reshape(b, c, hw))
    wq, wk, wv, wo, bo = _prep_weights(w_qkv, w_out, b_out)

    in_maps = []
    for core in range(8):
        bi, qi = core // 4, core % 4
        xb = np.ascontiguousarray(xf[bi])
        in_maps.append({
            "x": xb,
            "xq": np.ascontiguousarray(xb[:, QC * qi:QC * (qi + 1)]),
            "wq": wq, "wk": wk, "wv": wv, "wo": wo, "bo": bo,
        })

    nc = _get_nc()
    res = run_bass_kernel_spmd(nc, in_maps, core_ids=list(range(8)))
    y = np.empty((b, c, hw), np.float32)
    for core in range(8):
        bi, qi = core // 4, core % 4
        y[bi, :, QC * qi:QC * (qi + 1)] = res.results[core]["out"]
    return y.reshape(b, c, h, w)
